# revision 30
# baseline (speedup 1.0000x reference)
"""Trainium2 Bass kernel for a 3-layer GCN (nn_BaselineGCN).

Aggregate-first formulation (uses D~(HW) = (D~H)W):
  out_l = sigma( (D~ H_l) W_l + b_l ),  D~ = D^{-1/2}(A+I)D^{-1/2}

  - The gather table for layer l is T_l = dis (.) H_l (scaled by the SOURCE
    node's dis), 256-wide bf16 for every layer (512B gather descriptors).
  - Layer 1's table is dis (.) x == computable on the HOST: no z-phase, no
    layer-1 AllGather, gathers start at t~0.
  - Self-loop term: dis[i]*H[i] == T_own[i]; folded into the PSUM
    accumulation as one identity matmul per window.
  - Per dst-window epilogue: u = dis[w] (.) psw  (bf16), transpose via PE,
    u @ W_l, then T_{l+1}[w] = relu(dis[w] (.) (uW)) written to HBM and
    AllGathered (split A/B so the collective hides under the gathers).

Sharding: nodes partitioned across 8 cores by dst (6250/core, padded 6272);
edges sorted by (dst-window, src-half); int16 gather indices into two table
halves (A: 8*4096 rows, B: 8*2176 rows) so indices fit int16.
"""
import sys
import os

sys.path.insert(0, "/opt/trn_rl_repo")

import numpy as np

NC_CORES = 8
GMAX = 8  # max groups (=1024 indices) per dma_gather call (ucode limit)
GATH_BUFS = 9  # gather-tile pool depth
D = 256  # feature width of every gather table


def _cdiv(a, b):
    return (a + b - 1) // b


# ---------------------------------------------------------------------------
# Host-side preprocessing (same edge partitioning as before; indices are
# shared by all three layers)
# ---------------------------------------------------------------------------
def preprocess(edge_index, N):
    src = np.asarray(edge_index[0], dtype=np.int64)
    dst = np.asarray(edge_index[1], dtype=np.int64)
    deg = np.bincount(dst, minlength=N).astype(np.float32) + np.float32(1.0)
    dis = (np.float32(1.0) / np.sqrt(deg)).astype(np.float32)

    CH = N // NC_CORES
    NWIN = _cdiv(CH, 128)
    CHP = NWIN * 128
    # A as large as int16 gather indices allow (NC*HA <= 32768); B the rest
    NWA = min(NWIN - 1, 32768 // (NC_CORES * 128)) if NWIN > 1 else NWIN
    HA = NWA * 128
    HB = CHP - HA
    src_c = src // CH
    src_o = src % CH

    counts = np.zeros((NC_CORES, NWIN, 2), dtype=np.int64)
    percore = []
    for c in range(NC_CORES):
        sel = (dst >= c * CH) & (dst < (c + 1) * CH)
        sc, so = src_c[sel], src_o[sel]
        ed = dst[sel] - c * CH
        w = ed >> 7
        h = (so >= HA).astype(np.int64)
        eidx = np.where(h == 0, sc * HA + so, sc * HB + (so - HA))
        order = np.lexsort((ed, h, w))
        eidx, ed, w, h = eidx[order], ed[order], w[order], h[order]
        np.add.at(counts[c], (w, h), 1)
        percore.append((eidx, ed, w, h))

    G = _cdiv(counts, 128).max(axis=0)  # [NWIN, 2]

    import ml_dtypes

    # shared call schedule: per (window, half), gather calls of <=GMAX groups
    calls = []  # (wi, hi, g0, gc)
    for wi in range(NWIN):
        for hi in range(2):
            g0 = 0
            while g0 < G[wi, hi]:
                gc = min(GMAX, G[wi, hi] - g0)
                calls.append((wi, hi, g0, gc))
                g0 += gc

    cores = []
    for c in range(NC_CORES):
        eidx, ed, w, h = percore[c]
        idx_parts, dstl_parts = [], []
        pos = 0
        for wi in range(NWIN):
            for hi in range(2):
                n = counts[c, wi, hi]
                g = G[wi, hi]
                seg_idx = np.full(g * 128, -1, dtype=np.int16)
                seg_dstl = np.full(g * 128, 255.0, dtype=np.float32)
                if n:
                    seg_idx[:n] = eidx[pos:pos + n].astype(np.int16)
                    seg_dstl[:n] = (ed[pos:pos + n] - wi * 128).astype(np.float32)
                    pos += n
                idx_parts.append(seg_idx)
                dstl_parts.append(seg_dstl)
        idx_all = np.concatenate(idx_parts)
        dstl_all = np.concatenate(dstl_parts)
        TOT_G = len(idx_all) // 128

        # every call is fully valid: pad slots gather dummy rows (dstl=255
        # zeroes them in the one-hot). Constant per-call counts need no
        # per-call register loads and keep the pool rotation NaN-free.
        # Scatter the dummy rows across the table so they don't hammer a
        # single HBM row.
        pos3 = 0
        for wi in range(NWIN):
            for hi in range(2):
                seg = idx_all[pos3:pos3 + G[wi, hi] * 128]
                pos3 += G[wi, hi] * 128
                bad = seg < 0
                nb_ = int(bad.sum())
                if nb_:
                    hsz = NC_CORES * (HA if hi == 0 else HB)
                    seg[bad] = ((np.arange(nb_) * 8191 + wi * 127) %
                                hsz).astype(np.int16)

        # device layouts
        idx_tiled = np.tile(idx_all.reshape(-1, 16).T, (8, 1)).copy()
        dstl_tiled = np.ascontiguousarray(
            dstl_all.reshape(TOT_G, 128).T).astype(ml_dtypes.bfloat16)
        d = np.ones(CHP, np.float32)
        d[:CH] = dis[c * CH:(c + 1) * CH]
        dis_win = np.ascontiguousarray(d.reshape(NWIN, 128).T)
        cores.append(dict(idx=idx_tiled, dstl=dstl_tiled, dis_win=dis_win))
    return dis, G, cores, CH, NWIN, CHP, NWA, len(calls)


# ---------------------------------------------------------------------------
# Bass program
# ---------------------------------------------------------------------------
def build_program(DRS, G, NWIN, CHP, NWA, TOT_IDX, TOT_G, G_CAP, NCALLS,
                  biases_nonzero):
    """DRS: per-layer output dims [256, 256, 64]; every gather table is
    D=256 wide."""
    from concourse import bacc, bass, tile, mybir

    f32 = mybir.dt.float32
    bf16 = mybir.dt.bfloat16
    i16 = mybir.dt.int16
    ADD = mybir.AluOpType.add
    EQ = mybir.AluOpType.is_equal
    CPY = mybir.ActivationFunctionType.Copy
    RELU = mybir.ActivationFunctionType.Relu
    NL = len(DRS)
    HA = NWA * 128
    HB = CHP - HA

    nc = bacc.Bacc("TRN2", target_bir_lowering=False, debug=False,
                   enable_asserts=False, num_devices=NC_CORES,
                   num_swdge_queues=4, dynamic_dma_scratch_size=32768)

    # --- I/O tensors ---
    xsA_d = nc.dram_tensor("xsA", [NC_CORES * HA, D], bf16,
                           kind="ExternalInput")
    xsB_d = nc.dram_tensor("xsB", [NC_CORES * HB, D], bf16,
                           kind="ExternalInput")
    xso_d = nc.dram_tensor("xso", [CHP, D], bf16, kind="ExternalInput")
    W_d = [nc.dram_tensor(f"W{i}", [D, DRS[i]], bf16, kind="ExternalInput")
           for i in range(NL)]
    bias_d = [nc.dram_tensor(f"bias{i}", [128, DRS[i]], f32,
                             kind="ExternalInput") for i in range(NL)]
    idx_d = nc.dram_tensor("idx", [128, TOT_IDX // 16], i16,
                           kind="ExternalInput")
    cntv_d = nc.dram_tensor("cntv", [1, GMAX], mybir.dt.int32,
                            kind="ExternalInput")
    dstl_d = nc.dram_tensor("dstl", [128, TOT_G], bf16, kind="ExternalInput")
    iotag_d = nc.dram_tensor("iotag", [128, 128 * G_CAP], bf16,
                             kind="ExternalInput")
    dis_d = nc.dram_tensor("dis_win", [128, NWIN], f32, kind="ExternalInput")
    ident_d = nc.dram_tensor("ident", [128, 128], bf16, kind="ExternalInput")
    out_d = nc.dram_tensor("out", [CHP, DRS[-1]], f32, kind="ExternalOutput")

    with tile.TileContext(nc) as tc:
        with (
            tc.tile_pool(name="const", bufs=1) as constp,
            tc.tile_pool(name="wts", bufs=2) as wtsp,
            tc.tile_pool(name="town", bufs=4) as townp,
            tc.tile_pool(name="gath", bufs=GATH_BUFS) as gathp,
            tc.tile_pool(name="oh", bufs=7) as ohp,
            tc.tile_pool(name="epi", bufs=3) as epip,
            tc.tile_pool(name="ht", bufs=4) as htp,
            tc.tile_pool(name="psw", bufs=3, space="PSUM") as pswp,
            tc.tile_pool(name="ps2", bufs=2, space="PSUM") as ps2p,
            tc.tile_pool(name="pst", bufs=2, space="PSUM") as pstp,
            tc.tile_pool(name="dram", bufs=1, space="DRAM") as dramp,
        ):
            # --- persistent SBUF constants ---
            idx_t = constp.tile([128, TOT_IDX // 16], i16, tag="idx")
            nc.sync.dma_start(idx_t[:], idx_d[:])
            dstl_t = constp.tile([128, TOT_G], bf16, tag="dstl")
            nc.sync.dma_start(dstl_t[:], dstl_d[:])
            iotag_t = constp.tile([128, 128 * G_CAP], bf16, tag="iotag")
            nc.sync.dma_start(iotag_t[:], iotag_d[:])
            dis_t = constp.tile([128, NWIN], f32, tag="dis")
            nc.sync.dma_start(dis_t[:], dis_d[:])
            ident_t = constp.tile([128, 128], bf16, tag="ident")
            nc.sync.dma_start(ident_t[:], ident_d[:])
            # count registers: one per distinct call size, written exactly
            # once here (read-only afterwards -> no cross-call register deps)
            cntv_t = constp.tile([1, GMAX], mybir.dt.int32, tag="cntv")
            nc.sync.dma_start(cntv_t[:], cntv_d[:])
            cnt_regs = {}
            for j in range(GMAX):
                r = nc.gpsimd.alloc_register(f"gcnt{j}")
                nc.gpsimd.reg_load(r, cntv_t[0:1, j:j + 1])
                cnt_regs[(j + 1) * 128] = r
            bias_t = []
            for i in range(NL):
                if biases_nonzero[i]:
                    bt = constp.tile([128, DRS[i]], f32, tag=f"bias{i}")
                    nc.sync.dma_start(bt[:], bias_d[i][:])
                    bias_t.append(bt)
                else:
                    bias_t.append(None)

            # weight tiles (k-major, 2 tiles of [128, DRS[l]] each)
            wk = []
            for li in range(NL):
                wkl = []
                for k in range(D // 128):
                    wt_ = wtsp.tile([128, DRS[li]], bf16, tag=f"wk{li}_{k}",
                                    name=f"wk{li}_{k}")
                    nc.sync.dma_start(wt_[:], W_d[li][k * 128:(k + 1) * 128, :])
                    wkl.append(wt_)
                wk.append(wkl)

            # next-layer tables (own chunk + AllGathered full halves)
            t_own = [dramp.tile([CHP, D], bf16, tag=f"town{i}",
                                name=f"t_own{i}") for i in range(NL - 1)]
            t_fullA = [dramp.tile([NC_CORES * HA, D], bf16, tag=f"tfA{i}",
                                  addr_space="Shared", name=f"t_fullA{i}")
                       for i in range(NL - 1)]
            t_fullB = [dramp.tile([NC_CORES * HB, D], bf16, tag=f"tfB{i}",
                                  addr_space="Shared", name=f"t_fullB{i}")
                       for i in range(NL - 1)]

            RG = [list(range(NC_CORES))]

            def emit_ag(li, half):
                # AllGather own table chunk (li: produced-by layer index)
                if half == 0:
                    nc.gpsimd.collective_compute(
                        "AllGather", bass.mybir.AluOpType.bypass,
                        replica_groups=RG,
                        ins=[t_own[li][:HA, :]],
                        outs=[t_fullA[li].opt()])
                else:
                    nc.gpsimd.collective_compute(
                        "AllGather", bass.mybir.AluOpType.bypass,
                        replica_groups=RG,
                        ins=[t_own[li][HA:, :]],
                        outs=[t_fullB[li].opt()])

            # per-(window,half) idx-column offsets and per-window group
            # offsets (shared by all layers)
            off16 = {}
            goffs = []
            o16 = 0
            og = 0
            for w in range(NWIN):
                goffs.append(og)
                for h in range(2):
                    off16[(w, h)] = o16
                    o16 += int(G[w, h]) * 8
                og += int(G[w, 0]) + int(G[w, 1])

            def emit_half_calls(li, w, wt, half, tblA, tblB):
                gcnt = int(G[w, half])
                gbase = 0 if half == 0 else int(G[w, 0])
                tbl = tblA if half == 0 else tblB
                o = off16[(w, half)]
                g0 = 0
                while g0 < gcnt:
                    gc = min(GMAX, gcnt - g0)
                    nc.gpsimd.dma_gather(
                        wt[:, gbase + g0:gbase + g0 + gc, :],
                        tbl[:],
                        idx_t[:, o:o + gc * 8],
                        num_idxs=gc * 128,
                        num_idxs_reg=cnt_regs[gc * 128],
                        elem_size=D,
                        queue_num=w % 4,
                    )
                    o += gc * 8
                    g0 += gc

            def emit_compute(li, w, wt):
                Dr = DRS[li]
                selft = xso_d if li == 0 else t_own[li - 1]
                Gl, Gh = int(G[w, 0]), int(G[w, 1])
                Gt = Gl + Gh
                g_off = goffs[w]
                # one-hot: oh[p, j, g] = (dstl[p, g] == j)
                oh = ohp.tile([128, 128, Gt], bf16, tag="oh", name="oh")
                nc.vector.tensor_tensor(
                    oh[:],
                    dstl_t[:, g_off:g_off + Gt].unsqueeze(1)
                        .broadcast_to((128, 128, Gt)),
                    iotag_t[:].rearrange("p (j g) -> p j g", g=G_CAP)
                        [:, :, :Gt],
                    op=EQ,
                )
                # self term streamed through the same PSUM accumulation
                town = townp.tile([128, D], bf16, tag="town", name="town")
                nc.sync.dma_start(town[:], selft[w * 128:(w + 1) * 128, :])
                psw = pswp.tile([128, D], f32, tag="psw", name="psw")
                for g in range(Gt):
                    nc.tensor.matmul(psw[:], oh[:, :, g], wt[:, g, :],
                                     start=(g == 0), stop=False)
                nc.tensor.matmul(psw[:], ident_t[:], town[:],
                                 start=False, stop=True)
                # u = dis[w] (.) psw  (bf16) -> transpose -> @ W_li
                h2 = epip.tile([128, D], bf16, tag="h2", name="h2")
                nc.scalar.activation(h2[:], psw[:], CPY,
                                     scale=dis_t[:, w:w + 1])
                p2 = ps2p.tile([128, Dr], f32, tag="p2", name="p2")
                for k in range(D // 128):
                    pst = pstp.tile([128, 128], bf16, tag="pst", name="pst")
                    nc.tensor.transpose(
                        pst[:], h2[:, k * 128:(k + 1) * 128], ident_t[:])
                    hT = htp.tile([128, 128], bf16, tag="hT", name="hT")
                    nc.vector.tensor_copy(hT[:], pst[:])
                    nc.tensor.matmul(p2[:], hT[:], wk[li][k][:],
                                     start=(k == 0),
                                     stop=(k == D // 128 - 1))
                if li < NL - 1:
                    if bias_t[li] is not None:
                        tb = epip.tile([128, Dr], f32, tag="tb", name="tb")
                        nc.vector.tensor_tensor(tb[:], p2[:], bias_t[li][:],
                                                op=ADD)
                        src_ap = tb
                    else:
                        src_ap = p2
                    t2 = epip.tile([128, Dr], bf16, tag="t2", name="t2")
                    nc.scalar.activation(t2[:], src_ap[:], RELU,
                                         scale=dis_t[:, w:w + 1])
                    nc.sync.dma_start(t_own[li][w * 128:(w + 1) * 128, :],
                                      t2[:])
                else:
                    ot = epip.tile([128, Dr], f32, tag="ot", name="ot")
                    if bias_t[li] is not None:
                        nc.vector.tensor_tensor(ot[:], p2[:], bias_t[li][:],
                                                op=ADD)
                    else:
                        nc.vector.tensor_copy(ot[:], p2[:])
                    nc.sync.dma_start(out_d[w * 128:(w + 1) * 128, :], ot[:])

            AG_DELAY = 6  # windows of gather issue between AG-A dep and trigger
            PRE = 6       # windows whose A-half calls are emitted before the
                          # previous layer's B AllGather trigger
            for li in range(NL):
                tblA = xsA_d if li == 0 else t_fullA[li - 1]
                tblB = xsB_d if li == 0 else t_fullB[li - 1]
                # prefetch pass: A-half gathers of the first PRE windows keep
                # the queues busy while the previous layer's B half gathers
                # (AG-B trigger blocks the GpSimd stream until the last
                # window's table write lands)
                npre = min(PRE, NWIN) if li > 0 else 0
                wts = {}
                for w in range(npre):
                    wts[w] = gathp.tile([128, G_CAP, D], bf16, tag="gather",
                                        name="wt")
                    emit_half_calls(li, w, wts[w], 0, tblA, tblB)
                if li > 0:
                    emit_ag(li - 1, 1)
                for w in range(NWIN):
                    if w < npre:
                        wt = wts[w]
                    else:
                        wt = gathp.tile([128, G_CAP, D], bf16, tag="gather",
                                        name="wt")
                        emit_half_calls(li, w, wt, 0, tblA, tblB)
                    emit_half_calls(li, w, wt, 1, tblA, tblB)
                    emit_compute(li, w, wt)
                    if li < NL - 1 and w == min(NWA - 1 + AG_DELAY,
                                                NWIN - 1):
                        emit_ag(li, 0)
    nc.compile()
    return nc


# ---------------------------------------------------------------------------
# Entry point
# ---------------------------------------------------------------------------
def kernel(x, edge_index, W1, b1, W2, b2, W3, b3):
    from concourse.bass_utils import run_bass_kernel_spmd
    import ml_dtypes

    bfnp = ml_dtypes.bfloat16
    x = np.asarray(x, dtype=np.float32)
    Ws = [np.asarray(w, dtype=np.float32) for w in (W1, W2, W3)]
    bs = [np.asarray(b, dtype=np.float32) for b in (b1, b2, b3)]

    N, DIN = x.shape
    assert DIN == D
    DRS = [w.shape[1] for w in Ws]
    NL = 3

    dis, G, cores, CH, NWIN, CHP, NWA, NCALLS = preprocess(edge_index, N)
    HA = NWA * 128
    HB = CHP - HA
    TOT_IDX = cores[0]["idx"].shape[1] * 16
    TOT_G = cores[0]["dstl"].shape[1]
    G_CAP = int((G[:, 0] + G[:, 1]).max())
    biases_nonzero = [bool(np.any(b != 0)) for b in bs]

    nc = build_program(DRS, G, NWIN, CHP, NWA, TOT_IDX, TOT_G, G_CAP, NCALLS,
                       biases_nonzero)

    # host-side layer-1 table: xs = dis (.) x, packed into A/B halves
    xs = (dis[:, None] * x).astype(bfnp)
    xsA = np.zeros((NC_CORES * HA, D), bfnp)
    xsB = np.zeros((NC_CORES * HB, D), bfnp)
    for c in range(NC_CORES):
        na = min(HA, CH)
        xsA[c * HA:c * HA + na] = xs[c * CH:c * CH + na]
        nb = CH - na
        if nb > 0:
            xsB[c * HB:c * HB + nb] = xs[c * CH + na:(c + 1) * CH]

    ident = np.eye(128, dtype=bfnp)
    # iotag[p, j*G_CAP + g] = j
    iotag = np.tile(np.repeat(np.arange(128), G_CAP).astype(bfnp), (128, 1))
    in_maps = []
    for c in range(NC_CORES):
        xso = np.zeros((CHP, D), bfnp)
        xso[:CH] = xs[c * CH:(c + 1) * CH]
        m = {
            "xsA": xsA,
            "xsB": xsB,
            "xso": xso,
            "idx": cores[c]["idx"],
            "dstl": cores[c]["dstl"],
            "iotag": iotag,
            "dis_win": cores[c]["dis_win"],
            "ident": ident,
            "cntv": (np.arange(1, GMAX + 1, dtype=np.int32) * 128)[None, :],
        }
        for i in range(NL):
            m[f"W{i}"] = Ws[i].astype(bfnp)
            m[f"bias{i}"] = np.tile(bs[i][None, :], (128, 1))
        in_maps.append(m)

    trace = bool(int(os.environ.get("GCN_TRACE", "0")))
    res = run_bass_kernel_spmd(nc, in_maps, core_ids=list(range(NC_CORES)),
                               trace=trace)
    kernel.last_results = res
    out = np.concatenate([res.results[c]["out"][:CH] for c in range(NC_CORES)],
                         axis=0)
    return out.astype(np.float32)


# revision 31
# speedup vs baseline: 1.0920x; 1.0920x over previous
"""Trainium2 Bass kernel for a 3-layer GCN (nn_BaselineGCN).

Aggregate-first formulation (uses D~(HW) = (D~H)W):
  out_l = sigma( (D~ H_l) W_l + b_l ),  D~ = D^{-1/2}(A+I)D^{-1/2}

  - The gather table for layer l is T_l = dis (.) H_l (scaled by the SOURCE
    node's dis), 256-wide bf16 for every layer (512B gather descriptors).
  - Layer 1's table is dis (.) x == computable on the HOST: no z-phase, no
    layer-1 AllGather, gathers start at t~0.
  - Self-loop term: dis[i]*H[i] == T_own[i]; folded into the PSUM
    accumulation as one identity matmul per window.
  - Per dst-window epilogue: u = dis[w] (.) psw  (bf16), transpose via PE,
    u @ W_l, then T_{l+1}[w] = relu(dis[w] (.) (uW)) written to HBM and
    AllGathered (split A/B so the collective hides under the gathers).

Sharding: nodes partitioned across 8 cores by dst (6250/core, padded 6272);
edges sorted by (dst-window, src-half); int16 gather indices into two table
halves (A: 8*4096 rows, B: 8*2176 rows) so indices fit int16.
"""
import sys
import os

sys.path.insert(0, "/opt/trn_rl_repo")

import numpy as np

NC_CORES = 8
GMAX = 8  # max groups (=1024 indices) per dma_gather call (ucode limit)
GATH_BUFS = 9  # gather-tile pool depth
D = 256  # feature width of every gather table


def _cdiv(a, b):
    return (a + b - 1) // b


# ---------------------------------------------------------------------------
# Host-side preprocessing (same edge partitioning as before; indices are
# shared by all three layers)
# ---------------------------------------------------------------------------
def preprocess(edge_index, N):
    src = np.asarray(edge_index[0], dtype=np.int64)
    dst = np.asarray(edge_index[1], dtype=np.int64)
    deg = np.bincount(dst, minlength=N).astype(np.float32) + np.float32(1.0)
    dis = (np.float32(1.0) / np.sqrt(deg)).astype(np.float32)

    CH = N // NC_CORES
    NWIN = _cdiv(CH, 128)
    CHP = NWIN * 128
    # A as large as int16 gather indices allow (NC*HA <= 32768); B the rest
    NWA = min(NWIN - 1, 32768 // (NC_CORES * 128)) if NWIN > 1 else NWIN
    HA = NWA * 128
    HB = CHP - HA
    src_c = src // CH
    src_o = src % CH

    counts = np.zeros((NC_CORES, NWIN, 2), dtype=np.int64)
    percore = []
    for c in range(NC_CORES):
        sel = (dst >= c * CH) & (dst < (c + 1) * CH)
        sc, so = src_c[sel], src_o[sel]
        ed = dst[sel] - c * CH
        w = ed >> 7
        h = (so >= HA).astype(np.int64)
        eidx = np.where(h == 0, sc * HA + so, sc * HB + (so - HA))
        order = np.lexsort((ed, h, w))
        eidx, ed, w, h = eidx[order], ed[order], w[order], h[order]
        np.add.at(counts[c], (w, h), 1)
        percore.append((eidx, ed, w, h))

    G = _cdiv(counts, 128).max(axis=0)  # [NWIN, 2]

    import ml_dtypes

    # shared call schedule: per (window, half), gather calls of <=GMAX groups
    calls = []  # (wi, hi, g0, gc)
    for wi in range(NWIN):
        for hi in range(2):
            g0 = 0
            while g0 < G[wi, hi]:
                gc = min(GMAX, G[wi, hi] - g0)
                calls.append((wi, hi, g0, gc))
                g0 += gc

    cores = []
    for c in range(NC_CORES):
        eidx, ed, w, h = percore[c]
        idx_parts, dstl_parts = [], []
        pos = 0
        for wi in range(NWIN):
            for hi in range(2):
                n = counts[c, wi, hi]
                g = G[wi, hi]
                seg_idx = np.full(g * 128, -1, dtype=np.int16)
                seg_dstl = np.full(g * 128, 255.0, dtype=np.float32)
                if n:
                    seg_idx[:n] = eidx[pos:pos + n].astype(np.int16)
                    seg_dstl[:n] = (ed[pos:pos + n] - wi * 128).astype(np.float32)
                    pos += n
                idx_parts.append(seg_idx)
                dstl_parts.append(seg_dstl)
        idx_all = np.concatenate(idx_parts)
        dstl_all = np.concatenate(dstl_parts)
        TOT_G = len(idx_all) // 128

        # every call is fully valid: pad slots gather dummy rows (dstl=255
        # zeroes them in the one-hot). Constant per-call counts need no
        # per-call register loads and keep the pool rotation NaN-free.
        # Scatter the dummy rows across the table so they don't hammer a
        # single HBM row.
        pos3 = 0
        for wi in range(NWIN):
            for hi in range(2):
                seg = idx_all[pos3:pos3 + G[wi, hi] * 128]
                pos3 += G[wi, hi] * 128
                bad = seg < 0
                nb_ = int(bad.sum())
                if nb_:
                    hsz = NC_CORES * (HA if hi == 0 else HB)
                    seg[bad] = ((np.arange(nb_) * 8191 + wi * 127) %
                                hsz).astype(np.int16)

        # device layouts
        idx_tiled = np.tile(idx_all.reshape(-1, 16).T, (8, 1)).copy()
        dstl_tiled = np.ascontiguousarray(
            dstl_all.reshape(TOT_G, 128).T).astype(ml_dtypes.bfloat16)
        d = np.ones(CHP, np.float32)
        d[:CH] = dis[c * CH:(c + 1) * CH]
        dis_win = np.ascontiguousarray(d.reshape(NWIN, 128).T)
        cores.append(dict(idx=idx_tiled, dstl=dstl_tiled, dis_win=dis_win))
    return dis, G, cores, CH, NWIN, CHP, NWA, len(calls)


# ---------------------------------------------------------------------------
# Bass program
# ---------------------------------------------------------------------------
def build_program(DRS, G, NWIN, CHP, NWA, TOT_IDX, TOT_G, G_CAP, NCALLS,
                  biases_nonzero):
    """DRS: per-layer output dims [256, 256, 64]; every gather table is
    D=256 wide."""
    from concourse import bacc, bass, tile, mybir

    f32 = mybir.dt.float32
    bf16 = mybir.dt.bfloat16
    i16 = mybir.dt.int16
    ADD = mybir.AluOpType.add
    EQ = mybir.AluOpType.is_equal
    CPY = mybir.ActivationFunctionType.Copy
    RELU = mybir.ActivationFunctionType.Relu
    NL = len(DRS)
    HA = NWA * 128
    HB = CHP - HA

    nc = bacc.Bacc("TRN2", target_bir_lowering=False, debug=False,
                   enable_asserts=False, num_devices=NC_CORES,
                   num_swdge_queues=4, dynamic_dma_scratch_size=32768)

    # --- I/O tensors ---
    xsA_d = nc.dram_tensor("xsA", [NC_CORES * HA, D], bf16,
                           kind="ExternalInput")
    xsB_d = nc.dram_tensor("xsB", [NC_CORES * HB, D], bf16,
                           kind="ExternalInput")
    xso_d = nc.dram_tensor("xso", [CHP, D], bf16, kind="ExternalInput")
    W_d = [nc.dram_tensor(f"W{i}", [D, DRS[i]], bf16, kind="ExternalInput")
           for i in range(NL)]
    bias_d = [nc.dram_tensor(f"bias{i}", [128, DRS[i]], f32,
                             kind="ExternalInput") for i in range(NL)]
    idx_d = nc.dram_tensor("idx", [128, TOT_IDX // 16], i16,
                           kind="ExternalInput")
    cntv_d = nc.dram_tensor("cntv", [1, GMAX], mybir.dt.int32,
                            kind="ExternalInput")
    dstl_d = nc.dram_tensor("dstl", [128, TOT_G], bf16, kind="ExternalInput")
    iotag_d = nc.dram_tensor("iotag", [128, 128 * G_CAP], bf16,
                             kind="ExternalInput")
    dis_d = nc.dram_tensor("dis_win", [128, NWIN], f32, kind="ExternalInput")
    ident_d = nc.dram_tensor("ident", [128, 128], bf16, kind="ExternalInput")
    out_d = nc.dram_tensor("out", [CHP, DRS[-1]], f32, kind="ExternalOutput")

    with tile.TileContext(nc) as tc:
        with (
            tc.tile_pool(name="const", bufs=1) as constp,
            tc.tile_pool(name="wts", bufs=2) as wtsp,
            tc.tile_pool(name="town", bufs=4) as townp,
            tc.tile_pool(name="gath", bufs=GATH_BUFS) as gathp,
            tc.tile_pool(name="oh", bufs=7) as ohp,
            tc.tile_pool(name="epi", bufs=3) as epip,
            tc.tile_pool(name="ht", bufs=4) as htp,
            tc.tile_pool(name="psw", bufs=3, space="PSUM") as pswp,
            tc.tile_pool(name="ps2", bufs=2, space="PSUM") as ps2p,
            tc.tile_pool(name="pst", bufs=2, space="PSUM") as pstp,
            tc.tile_pool(name="dram", bufs=1, space="DRAM") as dramp,
        ):
            # --- persistent SBUF constants ---
            idx_t = constp.tile([128, TOT_IDX // 16], i16, tag="idx")
            nc.sync.dma_start(idx_t[:], idx_d[:])
            dstl_t = constp.tile([128, TOT_G], bf16, tag="dstl")
            nc.sync.dma_start(dstl_t[:], dstl_d[:])
            iotag_t = constp.tile([128, 128 * G_CAP], bf16, tag="iotag")
            nc.sync.dma_start(iotag_t[:], iotag_d[:])
            dis_t = constp.tile([128, NWIN], f32, tag="dis")
            nc.sync.dma_start(dis_t[:], dis_d[:])
            ident_t = constp.tile([128, 128], bf16, tag="ident")
            nc.sync.dma_start(ident_t[:], ident_d[:])
            # count registers: one per distinct call size, written exactly
            # once here (read-only afterwards -> no cross-call register deps)
            cntv_t = constp.tile([1, GMAX], mybir.dt.int32, tag="cntv")
            nc.sync.dma_start(cntv_t[:], cntv_d[:])
            cnt_regs = {}
            for j in range(GMAX):
                r = nc.gpsimd.alloc_register(f"gcnt{j}")
                nc.gpsimd.reg_load(r, cntv_t[0:1, j:j + 1])
                cnt_regs[(j + 1) * 128] = r
            bias_t = []
            for i in range(NL):
                if biases_nonzero[i]:
                    bt = constp.tile([128, DRS[i]], f32, tag=f"bias{i}")
                    nc.sync.dma_start(bt[:], bias_d[i][:])
                    bias_t.append(bt)
                else:
                    bias_t.append(None)

            # weight tiles (k-major, 2 tiles of [128, DRS[l]] each)
            wk = []
            for li in range(NL):
                wkl = []
                for k in range(D // 128):
                    wt_ = wtsp.tile([128, DRS[li]], bf16, tag=f"wk{li}_{k}",
                                    name=f"wk{li}_{k}")
                    nc.sync.dma_start(wt_[:], W_d[li][k * 128:(k + 1) * 128, :])
                    wkl.append(wt_)
                wk.append(wkl)

            # next-layer tables (own chunk + AllGathered full halves)
            t_own = [dramp.tile([CHP, D], bf16, tag=f"town{i}",
                                name=f"t_own{i}") for i in range(NL - 1)]
            t_fullA = [dramp.tile([NC_CORES * HA, D], bf16, tag=f"tfA{i}",
                                  addr_space="Shared", name=f"t_fullA{i}")
                       for i in range(NL - 1)]
            t_fullB = [dramp.tile([NC_CORES * HB, D], bf16, tag=f"tfB{i}",
                                  addr_space="Shared", name=f"t_fullB{i}")
                       for i in range(NL - 1)]

            RG = [list(range(NC_CORES))]

            def emit_ag(li, half):
                # AllGather own table chunk (li: produced-by layer index)
                if half == 0:
                    nc.gpsimd.collective_compute(
                        "AllGather", bass.mybir.AluOpType.bypass,
                        replica_groups=RG,
                        ins=[t_own[li][:HA, :]],
                        outs=[t_fullA[li].opt()])
                else:
                    nc.gpsimd.collective_compute(
                        "AllGather", bass.mybir.AluOpType.bypass,
                        replica_groups=RG,
                        ins=[t_own[li][HA:, :]],
                        outs=[t_fullB[li].opt()])

            # per-(window,half) idx-column offsets and per-window group
            # offsets (shared by all layers)
            off16 = {}
            goffs = []
            o16 = 0
            og = 0
            for w in range(NWIN):
                goffs.append(og)
                for h in range(2):
                    off16[(w, h)] = o16
                    o16 += int(G[w, h]) * 8
                og += int(G[w, 0]) + int(G[w, 1])

            qrr = [0]

            def emit_half_calls(li, w, wt, half, tblA, tblB):
                gcnt = int(G[w, half])
                gbase = 0 if half == 0 else int(G[w, 0])
                tbl = tblA if half == 0 else tblB
                o = off16[(w, half)]
                g0 = 0
                while g0 < gcnt:
                    gc = min(GMAX, gcnt - g0)
                    nc.gpsimd.dma_gather(
                        wt[:, gbase + g0:gbase + g0 + gc, :],
                        tbl[:],
                        idx_t[:, o:o + gc * 8],
                        num_idxs=gc * 128,
                        num_idxs_reg=cnt_regs[gc * 128],
                        elem_size=D,
                        queue_num=qrr[0] % 4,
                    )
                    qrr[0] += 1
                    o += gc * 8
                    g0 += gc

            def emit_compute(li, w, wt):
                Dr = DRS[li]
                selft = xso_d if li == 0 else t_own[li - 1]
                Gl, Gh = int(G[w, 0]), int(G[w, 1])
                Gt = Gl + Gh
                g_off = goffs[w]
                # one-hot: oh[p, j, g] = (dstl[p, g] == j)
                oh = ohp.tile([128, 128, Gt], bf16, tag="oh", name="oh")
                nc.vector.tensor_tensor(
                    oh[:],
                    dstl_t[:, g_off:g_off + Gt].unsqueeze(1)
                        .broadcast_to((128, 128, Gt)),
                    iotag_t[:].rearrange("p (j g) -> p j g", g=G_CAP)
                        [:, :, :Gt],
                    op=EQ,
                )
                # self term streamed through the same PSUM accumulation
                town = townp.tile([128, D], bf16, tag="town", name="town")
                nc.sync.dma_start(town[:], selft[w * 128:(w + 1) * 128, :])
                psw = pswp.tile([128, D], f32, tag="psw", name="psw")
                for g in range(Gt):
                    nc.tensor.matmul(psw[:], oh[:, :, g], wt[:, g, :],
                                     start=(g == 0), stop=False)
                nc.tensor.matmul(psw[:], ident_t[:], town[:],
                                 start=False, stop=True)
                # u = dis[w] (.) psw  (bf16) -> transpose -> @ W_li
                h2 = epip.tile([128, D], bf16, tag="h2", name="h2")
                nc.scalar.activation(h2[:], psw[:], CPY,
                                     scale=dis_t[:, w:w + 1])
                p2 = ps2p.tile([128, Dr], f32, tag="p2", name="p2")
                for k in range(D // 128):
                    pst = pstp.tile([128, 128], bf16, tag="pst", name="pst")
                    nc.tensor.transpose(
                        pst[:], h2[:, k * 128:(k + 1) * 128], ident_t[:])
                    hT = htp.tile([128, 128], bf16, tag="hT", name="hT")
                    nc.vector.tensor_copy(hT[:], pst[:])
                    nc.tensor.matmul(p2[:], hT[:], wk[li][k][:],
                                     start=(k == 0),
                                     stop=(k == D // 128 - 1))
                if li < NL - 1:
                    if bias_t[li] is not None:
                        tb = epip.tile([128, Dr], f32, tag="tb", name="tb")
                        nc.vector.tensor_tensor(tb[:], p2[:], bias_t[li][:],
                                                op=ADD)
                        src_ap = tb
                    else:
                        src_ap = p2
                    t2 = epip.tile([128, Dr], bf16, tag="t2", name="t2")
                    nc.scalar.activation(t2[:], src_ap[:], RELU,
                                         scale=dis_t[:, w:w + 1])
                    nc.sync.dma_start(t_own[li][w * 128:(w + 1) * 128, :],
                                      t2[:])
                else:
                    ot = epip.tile([128, Dr], f32, tag="ot", name="ot")
                    if bias_t[li] is not None:
                        nc.vector.tensor_tensor(ot[:], p2[:], bias_t[li][:],
                                                op=ADD)
                    else:
                        nc.vector.tensor_copy(ot[:], p2[:])
                    nc.sync.dma_start(out_d[w * 128:(w + 1) * 128, :], ot[:])

            AG_DELAY = 6  # windows of gather issue between AG-A dep and trigger
            PRE = 6       # windows whose A-half calls are emitted before the
                          # previous layer's B AllGather trigger
            for li in range(NL):
                tblA = xsA_d if li == 0 else t_fullA[li - 1]
                tblB = xsB_d if li == 0 else t_fullB[li - 1]
                # prefetch pass: A-half gathers of the first PRE windows keep
                # the queues busy while the previous layer's B half gathers
                # (AG-B trigger blocks the GpSimd stream until the last
                # window's table write lands)
                npre = min(PRE, NWIN) if li > 0 else 0
                wts = {}
                for w in range(npre):
                    wts[w] = gathp.tile([128, G_CAP, D], bf16, tag="gather",
                                        name="wt")
                    emit_half_calls(li, w, wts[w], 0, tblA, tblB)
                if li > 0:
                    emit_ag(li - 1, 1)
                for w in range(NWIN):
                    if w < npre:
                        wt = wts[w]
                    else:
                        wt = gathp.tile([128, G_CAP, D], bf16, tag="gather",
                                        name="wt")
                        emit_half_calls(li, w, wt, 0, tblA, tblB)
                    emit_half_calls(li, w, wt, 1, tblA, tblB)
                    emit_compute(li, w, wt)
                    if li < NL - 1 and w == min(NWA - 1 + AG_DELAY,
                                                NWIN - 1):
                        emit_ag(li, 0)
    nc.compile()
    return nc


# ---------------------------------------------------------------------------
# Entry point
# ---------------------------------------------------------------------------
def kernel(x, edge_index, W1, b1, W2, b2, W3, b3):
    from concourse.bass_utils import run_bass_kernel_spmd
    import ml_dtypes

    bfnp = ml_dtypes.bfloat16
    x = np.asarray(x, dtype=np.float32)
    Ws = [np.asarray(w, dtype=np.float32) for w in (W1, W2, W3)]
    bs = [np.asarray(b, dtype=np.float32) for b in (b1, b2, b3)]

    N, DIN = x.shape
    assert DIN == D
    DRS = [w.shape[1] for w in Ws]
    NL = 3

    dis, G, cores, CH, NWIN, CHP, NWA, NCALLS = preprocess(edge_index, N)
    HA = NWA * 128
    HB = CHP - HA
    TOT_IDX = cores[0]["idx"].shape[1] * 16
    TOT_G = cores[0]["dstl"].shape[1]
    G_CAP = int((G[:, 0] + G[:, 1]).max())
    biases_nonzero = [bool(np.any(b != 0)) for b in bs]

    nc = build_program(DRS, G, NWIN, CHP, NWA, TOT_IDX, TOT_G, G_CAP, NCALLS,
                       biases_nonzero)

    # host-side layer-1 table: xs = dis (.) x, packed into A/B halves
    xs = (dis[:, None] * x).astype(bfnp)
    xsA = np.zeros((NC_CORES * HA, D), bfnp)
    xsB = np.zeros((NC_CORES * HB, D), bfnp)
    for c in range(NC_CORES):
        na = min(HA, CH)
        xsA[c * HA:c * HA + na] = xs[c * CH:c * CH + na]
        nb = CH - na
        if nb > 0:
            xsB[c * HB:c * HB + nb] = xs[c * CH + na:(c + 1) * CH]

    ident = np.eye(128, dtype=bfnp)
    # iotag[p, j*G_CAP + g] = j
    iotag = np.tile(np.repeat(np.arange(128), G_CAP).astype(bfnp), (128, 1))
    in_maps = []
    for c in range(NC_CORES):
        xso = np.zeros((CHP, D), bfnp)
        xso[:CH] = xs[c * CH:(c + 1) * CH]
        m = {
            "xsA": xsA,
            "xsB": xsB,
            "xso": xso,
            "idx": cores[c]["idx"],
            "dstl": cores[c]["dstl"],
            "iotag": iotag,
            "dis_win": cores[c]["dis_win"],
            "ident": ident,
            "cntv": (np.arange(1, GMAX + 1, dtype=np.int32) * 128)[None, :],
        }
        for i in range(NL):
            m[f"W{i}"] = Ws[i].astype(bfnp)
            m[f"bias{i}"] = np.tile(bs[i][None, :], (128, 1))
        in_maps.append(m)

    trace = bool(int(os.environ.get("GCN_TRACE", "0")))
    res = run_bass_kernel_spmd(nc, in_maps, core_ids=list(range(NC_CORES)),
                               trace=trace)
    kernel.last_results = res
    out = np.concatenate([res.results[c]["out"][:CH] for c in range(NC_CORES)],
                         axis=0)
    return out.astype(np.float32)


# revision 33
# speedup vs baseline: 1.0934x; 1.0013x over previous
"""Trainium2 Bass kernel for a 3-layer GCN (nn_BaselineGCN).

Aggregate-first formulation (uses D~(HW) = (D~H)W):
  out_l = sigma( (D~ H_l) W_l + b_l ),  D~ = D^{-1/2}(A+I)D^{-1/2}

  - The gather table for layer l is T_l = dis (.) H_l (scaled by the SOURCE
    node's dis), 256-wide bf16 for every layer (512B gather descriptors).
  - Layer 1's table is dis (.) x == computable on the HOST: no z-phase, no
    layer-1 AllGather, gathers start at t~0.
  - Self-loop term: dis[i]*H[i] == T_own[i]; folded into the PSUM
    accumulation as one identity matmul per window.
  - Per dst-window epilogue: u = dis[w] (.) psw  (bf16), transpose via PE,
    u @ W_l, then T_{l+1}[w] = relu(dis[w] (.) (uW)) written to HBM and
    AllGathered (split A/B so the collective hides under the gathers).

Sharding: nodes partitioned across 8 cores by dst (6250/core, padded 6272);
edges sorted by (dst-window, src-half); int16 gather indices into two table
halves (A: 8*4096 rows, B: 8*2176 rows) so indices fit int16.
"""
import sys
import os

sys.path.insert(0, "/opt/trn_rl_repo")

import numpy as np

NC_CORES = 8
GMAX = 8  # max groups (=1024 indices) per dma_gather call (ucode limit)
GATH_BUFS = 9  # gather-tile pool depth
D = 256  # feature width of every gather table


def _cdiv(a, b):
    return (a + b - 1) // b


# ---------------------------------------------------------------------------
# Host-side preprocessing (same edge partitioning as before; indices are
# shared by all three layers)
# ---------------------------------------------------------------------------
def preprocess(edge_index, N):
    src = np.asarray(edge_index[0], dtype=np.int64)
    dst = np.asarray(edge_index[1], dtype=np.int64)
    deg = np.bincount(dst, minlength=N).astype(np.float32) + np.float32(1.0)
    dis = (np.float32(1.0) / np.sqrt(deg)).astype(np.float32)

    CH = N // NC_CORES
    NWIN = _cdiv(CH, 128)
    CHP = NWIN * 128
    # A as large as int16 gather indices allow (NC*HA <= 32768); B the rest
    NWA = min(NWIN - 1, 32768 // (NC_CORES * 128)) if NWIN > 1 else NWIN
    HA = NWA * 128
    HB = CHP - HA
    src_c = src // CH
    src_o = src % CH

    counts = np.zeros((NC_CORES, NWIN, 2), dtype=np.int64)
    percore = []
    for c in range(NC_CORES):
        sel = (dst >= c * CH) & (dst < (c + 1) * CH)
        sc, so = src_c[sel], src_o[sel]
        ed = dst[sel] - c * CH
        w = ed >> 7
        h = (so >= HA).astype(np.int64)
        eidx = np.where(h == 0, sc * HA + so, sc * HB + (so - HA))
        order = np.lexsort((ed, h, w))
        eidx, ed, w, h = eidx[order], ed[order], w[order], h[order]
        np.add.at(counts[c], (w, h), 1)
        percore.append((eidx, ed, w, h))

    G = _cdiv(counts, 128).max(axis=0)  # [NWIN, 2]

    import ml_dtypes

    # shared call schedule: per (window, half), gather calls of <=GMAX groups
    calls = []  # (wi, hi, g0, gc)
    for wi in range(NWIN):
        for hi in range(2):
            g0 = 0
            while g0 < G[wi, hi]:
                gc = min(GMAX, G[wi, hi] - g0)
                calls.append((wi, hi, g0, gc))
                g0 += gc

    cores = []
    for c in range(NC_CORES):
        eidx, ed, w, h = percore[c]
        idx_parts, dstl_parts = [], []
        pos = 0
        for wi in range(NWIN):
            for hi in range(2):
                n = counts[c, wi, hi]
                g = G[wi, hi]
                seg_idx = np.full(g * 128, -1, dtype=np.int16)
                seg_dstl = np.full(g * 128, 255.0, dtype=np.float32)
                if n:
                    seg_idx[:n] = eidx[pos:pos + n].astype(np.int16)
                    seg_dstl[:n] = (ed[pos:pos + n] - wi * 128).astype(np.float32)
                    pos += n
                idx_parts.append(seg_idx)
                dstl_parts.append(seg_dstl)
        idx_all = np.concatenate(idx_parts)
        dstl_all = np.concatenate(dstl_parts)
        TOT_G = len(idx_all) // 128

        # every call is fully valid: pad slots gather dummy rows (dstl=255
        # zeroes them in the one-hot). Constant per-call counts need no
        # per-call register loads and keep the pool rotation NaN-free.
        # Scatter the dummy rows across the table so they don't hammer a
        # single HBM row.
        pos3 = 0
        for wi in range(NWIN):
            for hi in range(2):
                seg = idx_all[pos3:pos3 + G[wi, hi] * 128]
                pos3 += G[wi, hi] * 128
                bad = seg < 0
                nb_ = int(bad.sum())
                if nb_:
                    hsz = NC_CORES * (HA if hi == 0 else HB)
                    seg[bad] = ((np.arange(nb_) * 8191 + wi * 127) %
                                hsz).astype(np.int16)

        # device layouts
        idx_tiled = np.tile(idx_all.reshape(-1, 16).T, (8, 1)).copy()
        dstl_tiled = np.ascontiguousarray(
            dstl_all.reshape(TOT_G, 128).T).astype(ml_dtypes.bfloat16)
        d = np.ones(CHP, np.float32)
        d[:CH] = dis[c * CH:(c + 1) * CH]
        dis_win = np.ascontiguousarray(d.reshape(NWIN, 128).T)
        cores.append(dict(idx=idx_tiled, dstl=dstl_tiled, dis_win=dis_win))
    return dis, G, cores, CH, NWIN, CHP, NWA, len(calls)


# ---------------------------------------------------------------------------
# Bass program
# ---------------------------------------------------------------------------
def build_program(DRS, G, NWIN, CHP, NWA, TOT_IDX, TOT_G, G_CAP, NCALLS,
                  biases_nonzero):
    """DRS: per-layer output dims [256, 256, 64]; every gather table is
    D=256 wide."""
    from concourse import bacc, bass, tile, mybir

    f32 = mybir.dt.float32
    bf16 = mybir.dt.bfloat16
    i16 = mybir.dt.int16
    ADD = mybir.AluOpType.add
    EQ = mybir.AluOpType.is_equal
    CPY = mybir.ActivationFunctionType.Copy
    RELU = mybir.ActivationFunctionType.Relu
    NL = len(DRS)
    HA = NWA * 128
    HB = CHP - HA

    nc = bacc.Bacc("TRN2", target_bir_lowering=False, debug=False,
                   enable_asserts=False, num_devices=NC_CORES,
                   num_swdge_queues=4, dynamic_dma_scratch_size=32768)

    # --- I/O tensors ---
    xsA_d = nc.dram_tensor("xsA", [NC_CORES * HA, D], bf16,
                           kind="ExternalInput")
    xsB_d = nc.dram_tensor("xsB", [NC_CORES * HB, D], bf16,
                           kind="ExternalInput")
    xso_d = nc.dram_tensor("xso", [CHP, D], bf16, kind="ExternalInput")
    W_d = [nc.dram_tensor(f"W{i}", [D, DRS[i]], bf16, kind="ExternalInput")
           for i in range(NL)]
    bias_d = [nc.dram_tensor(f"bias{i}", [128, DRS[i]], f32,
                             kind="ExternalInput") for i in range(NL)]
    idx_d = nc.dram_tensor("idx", [128, TOT_IDX // 16], i16,
                           kind="ExternalInput")
    cntv_d = nc.dram_tensor("cntv", [1, GMAX], mybir.dt.int32,
                            kind="ExternalInput")
    dstl_d = nc.dram_tensor("dstl", [128, TOT_G], bf16, kind="ExternalInput")
    iotag_d = nc.dram_tensor("iotag", [128, 128 * G_CAP], bf16,
                             kind="ExternalInput")
    dis_d = nc.dram_tensor("dis_win", [128, NWIN], f32, kind="ExternalInput")
    ident_d = nc.dram_tensor("ident", [128, 128], bf16, kind="ExternalInput")
    out_d = nc.dram_tensor("out", [CHP, DRS[-1]], f32, kind="ExternalOutput")

    with tile.TileContext(nc) as tc:
        with (
            tc.tile_pool(name="const", bufs=1) as constp,
            tc.tile_pool(name="wts", bufs=2) as wtsp,
            tc.tile_pool(name="town", bufs=4) as townp,
            tc.tile_pool(name="gath", bufs=GATH_BUFS) as gathp,
            tc.tile_pool(name="oh", bufs=7) as ohp,
            tc.tile_pool(name="epi", bufs=3) as epip,
            tc.tile_pool(name="ht", bufs=4) as htp,
            tc.tile_pool(name="psw", bufs=3, space="PSUM") as pswp,
            tc.tile_pool(name="ps2", bufs=2, space="PSUM") as ps2p,
            tc.tile_pool(name="pst", bufs=2, space="PSUM") as pstp,
            tc.tile_pool(name="dram", bufs=1, space="DRAM") as dramp,
        ):
            # --- persistent SBUF constants (small ones first; idx table in
            # window-aligned chunks so the first gathers depend only on the
            # first small DMA) ---
            dstl_t = constp.tile([128, TOT_G], bf16, tag="dstl")
            nc.sync.dma_start(dstl_t[:], dstl_d[:])
            dis_t = constp.tile([128, NWIN], f32, tag="dis")
            nc.sync.dma_start(dis_t[:], dis_d[:])
            ident_t = constp.tile([128, 128], bf16, tag="ident")
            nc.sync.dma_start(ident_t[:], ident_d[:])
            iotag_t = constp.tile([128, 128 * G_CAP], bf16, tag="iotag")
            nc.sync.dma_start(iotag_t[:], iotag_d[:])

            # idx chunk boundaries (in 16-col units) at window granularity
            wstart16 = []
            o16w = 0
            for w in range(NWIN):
                wstart16.append(o16w)
                o16w += (int(G[w, 0]) + int(G[w, 1])) * 8
            wstart16.append(o16w)
            bnd_w = [0] + [min(b, NWIN) for b in (4, 12, 24)
                           if b < NWIN] + [NWIN]
            idx_chunks = []  # (col_lo, col_hi, tile)
            for ci in range(len(bnd_w) - 1):
                lo, hi = wstart16[bnd_w[ci]], wstart16[bnd_w[ci + 1]]
                if hi <= lo:
                    continue
                t_ = constp.tile([128, hi - lo], i16, tag=f"idx{ci}")
                nc.sync.dma_start(t_[:], idx_d[:, lo:hi])
                idx_chunks.append((lo, hi, t_))

            def idx_slice(o, n):
                for lo, hi, t_ in idx_chunks:
                    if lo <= o and o + n <= hi:
                        return t_[:, o - lo:o - lo + n]
                raise AssertionError("idx slice spans chunks")
            # count registers: one per distinct call size, written exactly
            # once here (read-only afterwards -> no cross-call register deps)
            cntv_t = constp.tile([1, GMAX], mybir.dt.int32, tag="cntv")
            nc.sync.dma_start(cntv_t[:], cntv_d[:])
            cnt_regs = {}
            for j in range(GMAX):
                r = nc.gpsimd.alloc_register(f"gcnt{j}")
                nc.gpsimd.reg_load(r, cntv_t[0:1, j:j + 1])
                cnt_regs[(j + 1) * 128] = r
            bias_t = []
            for i in range(NL):
                if biases_nonzero[i]:
                    bt = constp.tile([128, DRS[i]], f32, tag=f"bias{i}")
                    nc.sync.dma_start(bt[:], bias_d[i][:])
                    bias_t.append(bt)
                else:
                    bias_t.append(None)

            # weight tiles (k-major, 2 tiles of [128, DRS[l]] each)
            wk = []
            for li in range(NL):
                wkl = []
                for k in range(D // 128):
                    wt_ = wtsp.tile([128, DRS[li]], bf16, tag=f"wk{li}_{k}",
                                    name=f"wk{li}_{k}")
                    nc.sync.dma_start(wt_[:], W_d[li][k * 128:(k + 1) * 128, :])
                    wkl.append(wt_)
                wk.append(wkl)

            # next-layer tables (own chunk + AllGathered full halves)
            t_own = [dramp.tile([CHP, D], bf16, tag=f"town{i}",
                                name=f"t_own{i}") for i in range(NL - 1)]
            t_fullA = [dramp.tile([NC_CORES * HA, D], bf16, tag=f"tfA{i}",
                                  addr_space="Shared", name=f"t_fullA{i}")
                       for i in range(NL - 1)]
            t_fullB = [dramp.tile([NC_CORES * HB, D], bf16, tag=f"tfB{i}",
                                  addr_space="Shared", name=f"t_fullB{i}")
                       for i in range(NL - 1)]

            RG = [list(range(NC_CORES))]

            def emit_ag(li, half):
                # AllGather own table chunk (li: produced-by layer index)
                if half == 0:
                    nc.gpsimd.collective_compute(
                        "AllGather", bass.mybir.AluOpType.bypass,
                        replica_groups=RG,
                        ins=[t_own[li][:HA, :]],
                        outs=[t_fullA[li].opt()])
                else:
                    nc.gpsimd.collective_compute(
                        "AllGather", bass.mybir.AluOpType.bypass,
                        replica_groups=RG,
                        ins=[t_own[li][HA:, :]],
                        outs=[t_fullB[li].opt()])

            # per-(window,half) idx-column offsets and per-window group
            # offsets (shared by all layers)
            off16 = {}
            goffs = []
            o16 = 0
            og = 0
            for w in range(NWIN):
                goffs.append(og)
                for h in range(2):
                    off16[(w, h)] = o16
                    o16 += int(G[w, h]) * 8
                og += int(G[w, 0]) + int(G[w, 1])

            qrr = [0]

            def emit_half_calls(li, w, wt, half, tblA, tblB):
                gcnt = int(G[w, half])
                gbase = 0 if half == 0 else int(G[w, 0])
                tbl = tblA if half == 0 else tblB
                o = off16[(w, half)]
                g0 = 0
                while g0 < gcnt:
                    gc = min(GMAX, gcnt - g0)
                    nc.gpsimd.dma_gather(
                        wt[:, gbase + g0:gbase + g0 + gc, :],
                        tbl[:],
                        idx_slice(o, gc * 8),
                        num_idxs=gc * 128,
                        num_idxs_reg=cnt_regs[gc * 128],
                        elem_size=D,
                        queue_num=qrr[0] % 4,
                    )
                    qrr[0] += 1
                    o += gc * 8
                    g0 += gc

            def emit_compute(li, w, wt):
                Dr = DRS[li]
                selft = xso_d if li == 0 else t_own[li - 1]
                Gl, Gh = int(G[w, 0]), int(G[w, 1])
                Gt = Gl + Gh
                g_off = goffs[w]
                # one-hot: oh[p, j, g] = (dstl[p, g] == j)
                oh = ohp.tile([128, 128, Gt], bf16, tag="oh", name="oh")
                nc.vector.tensor_tensor(
                    oh[:],
                    dstl_t[:, g_off:g_off + Gt].unsqueeze(1)
                        .broadcast_to((128, 128, Gt)),
                    iotag_t[:].rearrange("p (j g) -> p j g", g=G_CAP)
                        [:, :, :Gt],
                    op=EQ,
                )
                # self term streamed through the same PSUM accumulation
                town = townp.tile([128, D], bf16, tag="town", name="town")
                nc.sync.dma_start(town[:], selft[w * 128:(w + 1) * 128, :])
                psw = pswp.tile([128, D], f32, tag="psw", name="psw")
                for g in range(Gt):
                    nc.tensor.matmul(psw[:], oh[:, :, g], wt[:, g, :],
                                     start=(g == 0), stop=False)
                nc.tensor.matmul(psw[:], ident_t[:], town[:],
                                 start=False, stop=True)
                # u = dis[w] (.) psw  (bf16) -> transpose -> @ W_li
                h2 = epip.tile([128, D], bf16, tag="h2", name="h2")
                nc.scalar.activation(h2[:], psw[:], CPY,
                                     scale=dis_t[:, w:w + 1])
                p2 = ps2p.tile([128, Dr], f32, tag="p2", name="p2")
                for k in range(D // 128):
                    pst = pstp.tile([128, 128], bf16, tag="pst", name="pst")
                    nc.tensor.transpose(
                        pst[:], h2[:, k * 128:(k + 1) * 128], ident_t[:])
                    hT = htp.tile([128, 128], bf16, tag="hT", name="hT")
                    nc.vector.tensor_copy(hT[:], pst[:])
                    nc.tensor.matmul(p2[:], hT[:], wk[li][k][:],
                                     start=(k == 0),
                                     stop=(k == D // 128 - 1))
                if li < NL - 1:
                    if bias_t[li] is not None:
                        tb = epip.tile([128, Dr], f32, tag="tb", name="tb")
                        nc.vector.tensor_tensor(tb[:], p2[:], bias_t[li][:],
                                                op=ADD)
                        src_ap = tb
                    else:
                        src_ap = p2
                    t2 = epip.tile([128, Dr], bf16, tag="t2", name="t2")
                    nc.scalar.activation(t2[:], src_ap[:], RELU,
                                         scale=dis_t[:, w:w + 1])
                    nc.sync.dma_start(t_own[li][w * 128:(w + 1) * 128, :],
                                      t2[:])
                else:
                    ot = epip.tile([128, Dr], f32, tag="ot", name="ot")
                    if bias_t[li] is not None:
                        nc.vector.tensor_tensor(ot[:], p2[:], bias_t[li][:],
                                                op=ADD)
                    else:
                        nc.vector.tensor_copy(ot[:], p2[:])
                    nc.sync.dma_start(out_d[w * 128:(w + 1) * 128, :], ot[:])

            AG_DELAY = 6  # windows of gather issue between AG-A dep and trigger
            PRE = 6       # windows whose A-half calls are emitted before the
                          # previous layer's B AllGather trigger
            for li in range(NL):
                tblA = xsA_d if li == 0 else t_fullA[li - 1]
                tblB = xsB_d if li == 0 else t_fullB[li - 1]
                # prefetch pass: A-half gathers of the first PRE windows keep
                # the queues busy while the previous layer's B half gathers
                # (AG-B trigger blocks the GpSimd stream until the last
                # window's table write lands)
                npre = min(PRE, NWIN) if li > 0 else 0
                wts = {}
                for w in range(npre):
                    wts[w] = gathp.tile([128, G_CAP, D], bf16, tag="gather",
                                        name="wt")
                    emit_half_calls(li, w, wts[w], 0, tblA, tblB)
                if li > 0:
                    emit_ag(li - 1, 1)
                for w in range(NWIN):
                    if w < npre:
                        wt = wts[w]
                    else:
                        wt = gathp.tile([128, G_CAP, D], bf16, tag="gather",
                                        name="wt")
                        emit_half_calls(li, w, wt, 0, tblA, tblB)
                    emit_half_calls(li, w, wt, 1, tblA, tblB)
                    emit_compute(li, w, wt)
                    if li < NL - 1 and w == min(NWA - 1 + AG_DELAY,
                                                NWIN - 1):
                        emit_ag(li, 0)
    nc.compile()
    return nc


# ---------------------------------------------------------------------------
# Entry point
# ---------------------------------------------------------------------------
def kernel(x, edge_index, W1, b1, W2, b2, W3, b3):
    from concourse.bass_utils import run_bass_kernel_spmd
    import ml_dtypes

    bfnp = ml_dtypes.bfloat16
    x = np.asarray(x, dtype=np.float32)
    Ws = [np.asarray(w, dtype=np.float32) for w in (W1, W2, W3)]
    bs = [np.asarray(b, dtype=np.float32) for b in (b1, b2, b3)]

    N, DIN = x.shape
    assert DIN == D
    DRS = [w.shape[1] for w in Ws]
    NL = 3

    dis, G, cores, CH, NWIN, CHP, NWA, NCALLS = preprocess(edge_index, N)
    HA = NWA * 128
    HB = CHP - HA
    TOT_IDX = cores[0]["idx"].shape[1] * 16
    TOT_G = cores[0]["dstl"].shape[1]
    G_CAP = int((G[:, 0] + G[:, 1]).max())
    biases_nonzero = [bool(np.any(b != 0)) for b in bs]

    nc = build_program(DRS, G, NWIN, CHP, NWA, TOT_IDX, TOT_G, G_CAP, NCALLS,
                       biases_nonzero)

    # host-side layer-1 table: xs = dis (.) x, packed into A/B halves
    xs = (dis[:, None] * x).astype(bfnp)
    xsA = np.zeros((NC_CORES * HA, D), bfnp)
    xsB = np.zeros((NC_CORES * HB, D), bfnp)
    for c in range(NC_CORES):
        na = min(HA, CH)
        xsA[c * HA:c * HA + na] = xs[c * CH:c * CH + na]
        nb = CH - na
        if nb > 0:
            xsB[c * HB:c * HB + nb] = xs[c * CH + na:(c + 1) * CH]

    ident = np.eye(128, dtype=bfnp)
    # iotag[p, j*G_CAP + g] = j
    iotag = np.tile(np.repeat(np.arange(128), G_CAP).astype(bfnp), (128, 1))
    in_maps = []
    for c in range(NC_CORES):
        xso = np.zeros((CHP, D), bfnp)
        xso[:CH] = xs[c * CH:(c + 1) * CH]
        m = {
            "xsA": xsA,
            "xsB": xsB,
            "xso": xso,
            "idx": cores[c]["idx"],
            "dstl": cores[c]["dstl"],
            "iotag": iotag,
            "dis_win": cores[c]["dis_win"],
            "ident": ident,
            "cntv": (np.arange(1, GMAX + 1, dtype=np.int32) * 128)[None, :],
        }
        for i in range(NL):
            m[f"W{i}"] = Ws[i].astype(bfnp)
            m[f"bias{i}"] = np.tile(bs[i][None, :], (128, 1))
        in_maps.append(m)

    trace = bool(int(os.environ.get("GCN_TRACE", "0")))
    res = run_bass_kernel_spmd(nc, in_maps, core_ids=list(range(NC_CORES)),
                               trace=trace)
    kernel.last_results = res
    out = np.concatenate([res.results[c]["out"][:CH] for c in range(NC_CORES)],
                         axis=0)
    return out.astype(np.float32)


# revision 41
# speedup vs baseline: 1.2204x; 1.1161x over previous
"""Trainium2 Bass kernel for a 3-layer GCN (nn_BaselineGCN).

Aggregate-first formulation (uses D~(HW) = (D~H)W):
  out_l = sigma( (D~ H_l) W_l + b_l ),  D~ = D^{-1/2}(A+I)D^{-1/2}

  - The gather table for layer l is T_l = dis (.) H_l (scaled by the SOURCE
    node's dis), 256-wide bf16 for every layer (512B gather descriptors).
  - Layer 1's table is dis (.) x == computable on the HOST: no z-phase, no
    layer-1 AllGather, gathers start at t~0.
  - Self-loop term: dis[i]*H[i] == T_own[i]; folded into the PSUM
    accumulation as one identity matmul per window.
  - Per dst-window epilogue: u = dis[w] (.) psw  (bf16), transpose via PE,
    u @ W_l, then T_{l+1}[w] = relu(dis[w] (.) (uW)) written to HBM and
    AllGathered (split A/B so the collective hides under the gathers).

Sharding: nodes partitioned across 8 cores by dst (6250/core, padded 6272);
edges sorted by (dst-window, src-half); int16 gather indices into two table
halves (A: 8*4096 rows, B: 8*2176 rows) so indices fit int16.
"""
import sys
import os

sys.path.insert(0, "/opt/trn_rl_repo")

import numpy as np

NC_CORES = 8
GMAX = 8  # max groups (=1024 indices) per dma_gather call (ucode limit)
GATH_BUFS = 9  # gather-tile pool depth
D = 256  # feature width of every gather table


def _cdiv(a, b):
    return (a + b - 1) // b


# ---------------------------------------------------------------------------
# Host-side preprocessing (same edge partitioning as before; indices are
# shared by all three layers)
# ---------------------------------------------------------------------------
def preprocess(edge_index, N):
    src = np.asarray(edge_index[0], dtype=np.int64)
    dst = np.asarray(edge_index[1], dtype=np.int64)
    deg = np.bincount(dst, minlength=N).astype(np.float32) + np.float32(1.0)
    dis = (np.float32(1.0) / np.sqrt(deg)).astype(np.float32)

    CH = N // NC_CORES
    NWIN = _cdiv(CH, 128)
    CHP = NWIN * 128
    # A as large as int16 gather indices allow (NC*HA <= 32768); B the rest
    NWA = min(NWIN - 1, 32768 // (NC_CORES * 128)) if NWIN > 1 else NWIN
    HA = NWA * 128
    HB = CHP - HA
    src_c = src // CH
    src_o = src % CH

    counts = np.zeros((NC_CORES, NWIN, 2), dtype=np.int64)
    percore = []
    for c in range(NC_CORES):
        sel = (dst >= c * CH) & (dst < (c + 1) * CH)
        sc, so = src_c[sel], src_o[sel]
        ed = dst[sel] - c * CH
        w = ed >> 7
        h = (so >= HA).astype(np.int64)
        eidx = np.where(h == 0, sc * HA + so, sc * HB + (so - HA))
        order = np.lexsort((ed, h, w))
        eidx, ed, w, h = eidx[order], ed[order], w[order], h[order]
        np.add.at(counts[c], (w, h), 1)
        percore.append((eidx, ed, w, h))

    G = _cdiv(counts, 128).max(axis=0)  # [NWIN, 2]

    import ml_dtypes

    # shared call schedule: per (window, half), gather calls of <=GMAX groups
    calls = []  # (wi, hi, g0, gc)
    for wi in range(NWIN):
        for hi in range(2):
            g0 = 0
            while g0 < G[wi, hi]:
                gc = min(GMAX, G[wi, hi] - g0)
                calls.append((wi, hi, g0, gc))
                g0 += gc

    cores = []
    for c in range(NC_CORES):
        eidx, ed, w, h = percore[c]
        idx_parts, dstl_parts = [], []
        pos = 0
        for wi in range(NWIN):
            for hi in range(2):
                n = counts[c, wi, hi]
                g = G[wi, hi]
                seg_idx = np.full(g * 128, -1, dtype=np.int16)
                seg_dstl = np.full(g * 128, 255.0, dtype=np.float32)
                if n:
                    seg_idx[:n] = eidx[pos:pos + n].astype(np.int16)
                    seg_dstl[:n] = (ed[pos:pos + n] - wi * 128).astype(np.float32)
                    pos += n
                idx_parts.append(seg_idx)
                dstl_parts.append(seg_dstl)
        idx_all = np.concatenate(idx_parts)
        dstl_all = np.concatenate(dstl_parts)
        TOT_G = len(idx_all) // 128

        # every call is fully valid: pad slots gather dummy rows (dstl=255
        # zeroes them in the one-hot). Constant per-call counts need no
        # per-call register loads and keep the pool rotation NaN-free.
        # Scatter the dummy rows across the table so they don't hammer a
        # single HBM row.
        pos3 = 0
        for wi in range(NWIN):
            for hi in range(2):
                seg = idx_all[pos3:pos3 + G[wi, hi] * 128]
                pos3 += G[wi, hi] * 128
                bad = seg < 0
                nb_ = int(bad.sum())
                if nb_:
                    hsz = NC_CORES * (HA if hi == 0 else HB)
                    seg[bad] = ((np.arange(nb_) * 8191 + wi * 127) %
                                hsz).astype(np.int16)

        # device layouts
        idx_tiled = np.tile(idx_all.reshape(-1, 16).T, (8, 1)).copy()
        dstl_tiled = np.ascontiguousarray(
            dstl_all.reshape(TOT_G, 128).T).astype(ml_dtypes.bfloat16)
        d = np.ones(CHP, np.float32)
        d[:CH] = dis[c * CH:(c + 1) * CH]
        dis_win = np.ascontiguousarray(d.reshape(NWIN, 128).T)
        cores.append(dict(idx=idx_tiled, dstl=dstl_tiled, dis_win=dis_win))
    return dis, G, cores, CH, NWIN, CHP, NWA, len(calls)


# ---------------------------------------------------------------------------
# Bass program
# ---------------------------------------------------------------------------
def build_program(DRS, G, NWIN, CHP, NWA, TOT_IDX, TOT_G, G_CAP, NCALLS,
                  biases_nonzero):
    """DRS: per-layer output dims [256, 256, 64]; every gather table is
    D=256 wide."""
    from concourse import bacc, bass, tile, mybir

    f32 = mybir.dt.float32
    bf16 = mybir.dt.bfloat16
    f8 = mybir.dt.float8e4
    i16 = mybir.dt.int16
    ADD = mybir.AluOpType.add
    EQ = mybir.AluOpType.is_equal
    CPY = mybir.ActivationFunctionType.Copy
    RELU = mybir.ActivationFunctionType.Relu
    NL = len(DRS)
    HA = NWA * 128
    HB = CHP - HA

    nc = bacc.Bacc("TRN2", target_bir_lowering=False, debug=False,
                   enable_asserts=False, num_devices=NC_CORES,
                   num_swdge_queues=4, dynamic_dma_scratch_size=32768)

    # --- I/O tensors ---
    xsA_d = nc.dram_tensor("xsA", [NC_CORES * HA, D], bf16,
                           kind="ExternalInput")
    xsB_d = nc.dram_tensor("xsB", [NC_CORES * HB, D], bf16,
                           kind="ExternalInput")
    xso_d = nc.dram_tensor("xso", [CHP, D], bf16, kind="ExternalInput")
    W_d = [nc.dram_tensor(f"W{i}", [D, DRS[i]], bf16, kind="ExternalInput")
           for i in range(NL)]
    bias_d = [nc.dram_tensor(f"bias{i}", [128, DRS[i]], f32,
                             kind="ExternalInput") for i in range(NL)]
    idx_d = nc.dram_tensor("idx", [128, TOT_IDX // 16], i16,
                           kind="ExternalInput")
    cntv_d = nc.dram_tensor("cntv", [1, GMAX], mybir.dt.int32,
                            kind="ExternalInput")
    dstl_d = nc.dram_tensor("dstl", [128, TOT_G], bf16, kind="ExternalInput")
    iotag_d = nc.dram_tensor("iotag", [128, 128 * G_CAP], bf16,
                             kind="ExternalInput")
    dis_d = nc.dram_tensor("dis_win", [128, NWIN], f32, kind="ExternalInput")
    ident_d = nc.dram_tensor("ident", [128, 128], bf16, kind="ExternalInput")
    identf8_d = nc.dram_tensor("identf8", [128, 128], f8,
                               kind="ExternalInput")
    out_d = nc.dram_tensor("out", [CHP, DRS[-1]], f32, kind="ExternalOutput")

    with tile.TileContext(nc) as tc:
        with (
            tc.tile_pool(name="const", bufs=1) as constp,
            tc.tile_pool(name="wts", bufs=2) as wtsp,
            tc.tile_pool(name="town", bufs=4) as townp,
            tc.tile_pool(name="gath", bufs=GATH_BUFS) as gathp,
            tc.tile_pool(name="oh", bufs=7) as ohp,
            tc.tile_pool(name="epi", bufs=3) as epip,
            tc.tile_pool(name="ht", bufs=4) as htp,
            tc.tile_pool(name="psw", bufs=3, space="PSUM") as pswp,
            tc.tile_pool(name="ps2", bufs=2, space="PSUM") as ps2p,
            tc.tile_pool(name="pst", bufs=2, space="PSUM") as pstp,
            tc.tile_pool(name="dram", bufs=1, space="DRAM") as dramp,
        ):
            # --- persistent SBUF constants (small ones first; idx table in
            # window-aligned chunks so the first gathers depend only on the
            # first small DMA) ---
            dstl_t = constp.tile([128, TOT_G], bf16, tag="dstl")
            nc.sync.dma_start(dstl_t[:], dstl_d[:])
            dis_t = constp.tile([128, NWIN], f32, tag="dis")
            nc.sync.dma_start(dis_t[:], dis_d[:])
            ident_t = constp.tile([128, 128], bf16, tag="ident")
            nc.sync.dma_start(ident_t[:], ident_d[:])
            identf8_t = constp.tile([128, 128], f8, tag="identf8")
            nc.sync.dma_start(identf8_t[:], identf8_d[:])
            iotag_t = constp.tile([128, 128 * G_CAP], bf16, tag="iotag")
            nc.sync.dma_start(iotag_t[:], iotag_d[:])

            # idx chunk boundaries (in 16-col units) at window granularity
            wstart16 = []
            o16w = 0
            for w in range(NWIN):
                wstart16.append(o16w)
                o16w += (int(G[w, 0]) + int(G[w, 1])) * 8
            wstart16.append(o16w)
            bnd_w = [0] + [min(b, NWIN) for b in (4, 12, 24)
                           if b < NWIN] + [NWIN]
            idx_chunks = []  # (col_lo, col_hi, tile)
            for ci in range(len(bnd_w) - 1):
                lo, hi = wstart16[bnd_w[ci]], wstart16[bnd_w[ci + 1]]
                if hi <= lo:
                    continue
                t_ = constp.tile([128, hi - lo], i16, tag=f"idx{ci}")
                nc.sync.dma_start(t_[:], idx_d[:, lo:hi])
                idx_chunks.append((lo, hi, t_))

            def idx_slice(o, n):
                for lo, hi, t_ in idx_chunks:
                    if lo <= o and o + n <= hi:
                        return t_[:, o - lo:o - lo + n]
                raise AssertionError("idx slice spans chunks")
            # count registers: one per distinct call size, written exactly
            # once here (read-only afterwards -> no cross-call register deps)
            cntv_t = constp.tile([1, GMAX], mybir.dt.int32, tag="cntv")
            nc.sync.dma_start(cntv_t[:], cntv_d[:])
            cnt_regs = {}
            for j in range(GMAX):
                r = nc.gpsimd.alloc_register(f"gcnt{j}")
                nc.gpsimd.reg_load(r, cntv_t[0:1, j:j + 1])
                cnt_regs[(j + 1) * 128] = r
            bias_t = []
            for i in range(NL):
                if biases_nonzero[i]:
                    bt = constp.tile([128, DRS[i]], f32, tag=f"bias{i}")
                    nc.sync.dma_start(bt[:], bias_d[i][:])
                    bias_t.append(bt)
                else:
                    bias_t.append(None)

            # weight tiles (k-major, 2 tiles of [128, DRS[l]] each)
            wk = []
            for li in range(NL):
                wkl = []
                for k in range(D // 128):
                    wt_ = wtsp.tile([128, DRS[li]], bf16, tag=f"wk{li}_{k}",
                                    name=f"wk{li}_{k}")
                    nc.sync.dma_start(wt_[:], W_d[li][k * 128:(k + 1) * 128, :])
                    wkl.append(wt_)
                wk.append(wkl)

            # next-layer tables (own chunk + AllGathered full halves), fp8:
            # halves collective + gather bytes; ~17 positive terms per sum
            # keep the quantization error ~1% at the output
            t_own = [dramp.tile([CHP, D], f8, tag=f"town{i}",
                                name=f"t_own{i}") for i in range(NL - 1)]
            t_fullA = [dramp.tile([NC_CORES * HA, D], f8, tag=f"tfA{i}",
                                  addr_space="Shared", name=f"t_fullA{i}")
                       for i in range(NL - 1)]
            t_fullB = [dramp.tile([NC_CORES * HB, D], f8, tag=f"tfB{i}",
                                  addr_space="Shared", name=f"t_fullB{i}")
                       for i in range(NL - 1)]

            RG = [list(range(NC_CORES))]

            def emit_ag(li, half):
                # AllGather own table chunk (li: produced-by layer index)
                if half == 0:
                    nc.gpsimd.collective_compute(
                        "AllGather", bass.mybir.AluOpType.bypass,
                        replica_groups=RG,
                        ins=[t_own[li][:HA, :]],
                        outs=[t_fullA[li].opt()])
                else:
                    nc.gpsimd.collective_compute(
                        "AllGather", bass.mybir.AluOpType.bypass,
                        replica_groups=RG,
                        ins=[t_own[li][HA:, :]],
                        outs=[t_fullB[li].opt()])

            # per-(window,half) idx-column offsets and per-window group
            # offsets (shared by all layers)
            off16 = {}
            goffs = []
            o16 = 0
            og = 0
            for w in range(NWIN):
                goffs.append(og)
                for h in range(2):
                    off16[(w, h)] = o16
                    o16 += int(G[w, h]) * 8
                og += int(G[w, 0]) + int(G[w, 1])

            qrr = [0]

            def emit_half_calls(li, w, wt, half, tblA, tblB):
                gcnt = int(G[w, half])
                gbase = 0 if half == 0 else int(G[w, 0])
                tbl = tblA if half == 0 else tblB
                o = off16[(w, half)]
                g0 = 0
                while g0 < gcnt:
                    gc = min(GMAX, gcnt - g0)
                    nc.gpsimd.dma_gather(
                        wt[:, gbase + g0:gbase + g0 + gc, :],
                        tbl[:],
                        idx_slice(o, gc * 8),
                        num_idxs=gc * 128,
                        num_idxs_reg=cnt_regs[gc * 128],
                        elem_size=D,
                        queue_num=qrr[0] % 4,
                    )
                    qrr[0] += 1
                    o += gc * 8
                    g0 += gc

            def emit_compute(li, w, wt):
                Dr = DRS[li]
                tdt = bf16 if li == 0 else f8
                idt = ident_t if li == 0 else identf8_t
                selft = xso_d if li == 0 else t_own[li - 1]
                Gl, Gh = int(G[w, 0]), int(G[w, 1])
                Gt = Gl + Gh
                g_off = goffs[w]
                # one-hot: oh[p, j, g] = (dstl[p, g] == j)
                oh = ohp.tile([128, 128, Gt], tdt, tag="oh", name="oh")
                nc.vector.tensor_tensor(
                    oh[:],
                    dstl_t[:, g_off:g_off + Gt].unsqueeze(1)
                        .broadcast_to((128, 128, Gt)),
                    iotag_t[:].rearrange("p (j g) -> p j g", g=G_CAP)
                        [:, :, :Gt],
                    op=EQ,
                )
                # self term streamed through the same PSUM accumulation
                town = townp.tile([128, D], tdt, tag="town", name="town")
                nc.sync.dma_start(town[:], selft[w * 128:(w + 1) * 128, :])
                psw = pswp.tile([128, D], f32, tag="psw", name="psw")
                for g in range(Gt):
                    nc.tensor.matmul(psw[:], oh[:, :, g], wt[:, g, :],
                                     start=(g == 0), stop=False)
                nc.tensor.matmul(psw[:], idt[:], town[:],
                                 start=False, stop=True)
                # u = dis[w] (.) psw  (bf16) -> transpose -> @ W_li
                h2 = epip.tile([128, D], bf16, tag="h2", name="h2")
                nc.scalar.activation(h2[:], psw[:], CPY,
                                     scale=dis_t[:, w:w + 1])
                p2 = ps2p.tile([128, Dr], f32, tag="p2", name="p2")
                for k in range(D // 128):
                    pst = pstp.tile([128, 128], bf16, tag="pst", name="pst")
                    nc.tensor.transpose(
                        pst[:], h2[:, k * 128:(k + 1) * 128], ident_t[:])
                    hT = htp.tile([128, 128], bf16, tag="hT", name="hT")
                    nc.vector.tensor_copy(hT[:], pst[:])
                    nc.tensor.matmul(p2[:], hT[:], wk[li][k][:],
                                     start=(k == 0),
                                     stop=(k == D // 128 - 1))
                if li < NL - 1:
                    if bias_t[li] is not None:
                        tb = epip.tile([128, Dr], f32, tag="tb", name="tb")
                        nc.vector.tensor_tensor(tb[:], p2[:], bias_t[li][:],
                                                op=ADD)
                        src_ap = tb
                    else:
                        src_ap = p2
                    t2 = epip.tile([128, Dr], f8, tag="t2", name="t2")
                    nc.scalar.activation(t2[:], src_ap[:], RELU,
                                         scale=dis_t[:, w:w + 1])
                    nc.sync.dma_start(t_own[li][w * 128:(w + 1) * 128, :],
                                      t2[:])
                else:
                    ot = epip.tile([128, Dr], f32, tag="ot", name="ot")
                    if bias_t[li] is not None:
                        nc.vector.tensor_tensor(ot[:], p2[:], bias_t[li][:],
                                                op=ADD)
                    else:
                        nc.vector.tensor_copy(ot[:], p2[:])
                    nc.sync.dma_start(out_d[w * 128:(w + 1) * 128, :], ot[:])

            AG_DELAY = 6  # windows of gather issue between AG-A dep and trigger
            PRE = 6       # windows whose A-half calls are emitted before the
                          # previous layer's B AllGather trigger
            for li in range(NL):
                tblA = xsA_d if li == 0 else t_fullA[li - 1]
                tblB = xsB_d if li == 0 else t_fullB[li - 1]
                # prefetch pass: A-half gathers of the first PRE windows keep
                # the queues busy while the previous layer's B half gathers
                # (AG-B trigger blocks the GpSimd stream until the last
                # window's table write lands)
                tdt = bf16 if li == 0 else f8
                npre = min(PRE, NWIN) if li > 0 else 0
                wts = {}
                for w in range(npre):
                    wts[w] = gathp.tile([128, G_CAP, D], tdt, tag="gather",
                                        name="wt")
                    emit_half_calls(li, w, wts[w], 0, tblA, tblB)
                if li > 0:
                    emit_ag(li - 1, 1)
                for w in range(NWIN):
                    if w < npre:
                        wt = wts[w]
                    else:
                        wt = gathp.tile([128, G_CAP, D], tdt, tag="gather",
                                        name="wt")
                        emit_half_calls(li, w, wt, 0, tblA, tblB)
                    emit_half_calls(li, w, wt, 1, tblA, tblB)
                    emit_compute(li, w, wt)
                    if li < NL - 1 and w == min(NWA - 1 + AG_DELAY,
                                                NWIN - 1):
                        emit_ag(li, 0)
    nc.compile()
    return nc


# ---------------------------------------------------------------------------
# Entry point
# ---------------------------------------------------------------------------
def kernel(x, edge_index, W1, b1, W2, b2, W3, b3):
    from concourse.bass_utils import run_bass_kernel_spmd
    import ml_dtypes

    bfnp = ml_dtypes.bfloat16
    x = np.asarray(x, dtype=np.float32)
    Ws = [np.asarray(w, dtype=np.float32) for w in (W1, W2, W3)]
    bs = [np.asarray(b, dtype=np.float32) for b in (b1, b2, b3)]

    N, DIN = x.shape
    assert DIN == D
    DRS = [w.shape[1] for w in Ws]
    NL = 3

    dis, G, cores, CH, NWIN, CHP, NWA, NCALLS = preprocess(edge_index, N)
    HA = NWA * 128
    HB = CHP - HA
    TOT_IDX = cores[0]["idx"].shape[1] * 16
    TOT_G = cores[0]["dstl"].shape[1]
    G_CAP = int((G[:, 0] + G[:, 1]).max())
    biases_nonzero = [bool(np.any(b != 0)) for b in bs]

    nc = build_program(DRS, G, NWIN, CHP, NWA, TOT_IDX, TOT_G, G_CAP, NCALLS,
                       biases_nonzero)

    # host-side layer-1 table: xs = dis (.) x, packed into A/B halves
    xs = (dis[:, None] * x).astype(bfnp)
    xsA = np.zeros((NC_CORES * HA, D), bfnp)
    xsB = np.zeros((NC_CORES * HB, D), bfnp)
    for c in range(NC_CORES):
        na = min(HA, CH)
        xsA[c * HA:c * HA + na] = xs[c * CH:c * CH + na]
        nb = CH - na
        if nb > 0:
            xsB[c * HB:c * HB + nb] = xs[c * CH + na:(c + 1) * CH]

    ident = np.eye(128, dtype=bfnp)
    # iotag[p, j*G_CAP + g] = j
    iotag = np.tile(np.repeat(np.arange(128), G_CAP).astype(bfnp), (128, 1))
    in_maps = []
    for c in range(NC_CORES):
        xso = np.zeros((CHP, D), bfnp)
        xso[:CH] = xs[c * CH:(c + 1) * CH]
        m = {
            "xsA": xsA,
            "xsB": xsB,
            "xso": xso,
            "idx": cores[c]["idx"],
            "dstl": cores[c]["dstl"],
            "iotag": iotag,
            "dis_win": cores[c]["dis_win"],
            "ident": ident,
            "identf8": np.eye(128, dtype=ml_dtypes.float8_e4m3),
            "cntv": (np.arange(1, GMAX + 1, dtype=np.int32) * 128)[None, :],
        }
        for i in range(NL):
            m[f"W{i}"] = Ws[i].astype(bfnp)
            m[f"bias{i}"] = np.tile(bs[i][None, :], (128, 1))
        in_maps.append(m)

    trace = bool(int(os.environ.get("GCN_TRACE", "0")))
    res = run_bass_kernel_spmd(nc, in_maps, core_ids=list(range(NC_CORES)),
                               trace=trace)
    kernel.last_results = res
    out = np.concatenate([res.results[c]["out"][:CH] for c in range(NC_CORES)],
                         axis=0)
    return out.astype(np.float32)
